# revision 1
# baseline (speedup 1.0000x reference)
"""Cox partial-likelihood (DeepSurv) loss on 8 TRN2 NeuronCores.

Math: P_exp_sum[i] = sum_j P_exp[j] * (T[i] < T[j]); loss is a scalar
reduction over log(P_exp / (P_exp_sum + eps)) masked by events.

Device does the O(N^2) risk-set sum, data-parallel over rows:
core c owns i in [c*2048, (c+1)*2048). For each 128-wide j-chunk an
engine builds a [128 j, 2048 i] comparison tile and the PE contracts
over j with stationary per-chunk weight columns, accumulating into
PSUM over all 128 chunks:

- 3 of 4 chunks on the DVE: mask = (T_i < T_j) via is_lt with a
  per-partition scalar -> exact {0,1} bf16 (fp32 compare, ties exact).
  Weights are [hi(P_exp_j), lo(P_exp_j)] (bf16 hi/lo split -> ~17-bit
  mantissa).
- 1 of 4 chunks on the ACT engine (load-balancing the mask work):
  smask = Sign(T_j - T_i) in {-1, 0, +1}, consumed by the same matmul
  with weights hi/lo of 0.5*P_exp_j. The sign trick yields
  0.5*(G_i - L_i); the host adds 0.5*S_act and subtracts the exact
  tie-sum 0.5*Eq_i (computed via np.unique; Eq includes j == i) to
  recover G_i = sum over strictly-greater j.

Host does the remaining O(N) epilogue exactly in fp32.
"""

import numpy as np
import ml_dtypes

N = 16384
NCORES = 8
LI = N // NCORES          # rows per core
KC = N // 128             # 128-wide j-chunks
NB = LI // 512            # psum banks per core
EPS = 1e-6

# j-chunks assigned to the ACT engine (Sign path); rest on DVE.
# 1/4 is HW-verified end-to-end (rel err 1.04e-7, ~112-120us). 1/3
# measured up to 4% faster and is compiled+sim-verified, but its final
# HW correctness run did not complete in-session; keeping 1/4.
ACT_EVERY = 4
ACT_PHASE = 2


def _act_chunks():
    return [k for k in range(KC) if k % ACT_EVERY == ACT_PHASE]


_prog_cache = {}


def _build_program(reps=1):
    if reps in _prog_cache:
        return _prog_cache[reps]
    import concourse.bacc as bacc
    import concourse.tile as tile
    import concourse.mybir as mybir

    act_set = set(_act_chunks())
    nc = bacc.Bacc(
        "TRN2", target_bir_lowering=False, debug=False, num_devices=NCORES
    )
    tib = nc.dram_tensor("tib", [128, LI], mybir.dt.float32, kind="ExternalInput").ap()
    tj = nc.dram_tensor("tj", [128, KC], mybir.dt.float32, kind="ExternalInput").ap()
    w = nc.dram_tensor("w", [128, 2 * KC], mybir.dt.bfloat16, kind="ExternalInput").ap()
    out = nc.dram_tensor("out", [2, LI], mybir.dt.float32, kind="ExternalOutput").ap()

    with tile.TileContext(nc) as tc:
        with (
            tc.tile_pool(name="const", bufs=1) as cpool,
            tc.tile_pool(name="mask", bufs=32) as mpool,
            tc.tile_pool(name="psum", bufs=1, space="PSUM") as ppool,
            tc.tile_pool(name="res", bufs=1) as rpool,
        ):
            tib_s = cpool.tile([128, LI], mybir.dt.float32)
            nc.sync.dma_start(tib_s[:], tib[:])
            tj_s = cpool.tile([128, KC], mybir.dt.float32)
            nc.sync.dma_start(tj_s[:], tj[:])
            w_s = cpool.tile([128, 2 * KC], mybir.dt.bfloat16)
            nc.sync.dma_start(w_s[:], w[:])

            psums = [
                ppool.tile([2, 512], mybir.dt.float32, name=f"psum{b}", tag=f"psum{b}")
                for b in range(NB)
            ]
            res = rpool.tile([2, LI], mybir.dt.float32)
            for _ in range(reps):
                for k in range(KC):
                    mask = mpool.tile(
                        [128, LI], mybir.dt.bfloat16, name="mask", tag="mask"
                    )
                    if k in act_set:
                        nc.scalar.activation(
                            mask[:],
                            tib_s[:],
                            mybir.ActivationFunctionType.Sign,
                            bias=tj_s[:, k : k + 1],
                            scale=-1.0,
                        )
                    else:
                        nc.vector.tensor_scalar(
                            mask[:],
                            tib_s[:],
                            tj_s[:, k : k + 1],
                            None,
                            mybir.AluOpType.is_lt,
                        )
                    for b in range(NB):
                        nc.tensor.matmul(
                            psums[b][:],
                            w_s[:, 2 * k : 2 * k + 2],
                            mask[:, 512 * b : 512 * (b + 1)],
                            start=(k == 0),
                            stop=(k == KC - 1),
                        )
                for b in range(NB):
                    nc.vector.tensor_copy(res[:, 512 * b : 512 * (b + 1)], psums[b][:])
            nc.sync.dma_start(out[:], res[:])
    nc.compile()
    _prog_cache[reps] = nc
    return nc


def _hi_lo(x):
    hi = x.astype(ml_dtypes.bfloat16)
    lo = (x - hi.astype(np.float32)).astype(ml_dtypes.bfloat16)
    return hi, lo


def _make_in_maps(P_risk, T):
    P_exp = np.exp(P_risk.astype(np.float32))
    # DVE chunks: weights = (hi, lo) of P_exp; ACT chunks: of 0.5*P_exp
    # (the sign mask contributes G - L; the 0.5 folds the averaging in).
    wfull = P_exp.copy()
    act_j = np.zeros(N, dtype=bool)
    for k in _act_chunks():
        act_j[k * 128 : (k + 1) * 128] = True
    wfull[act_j] *= np.float32(0.5)
    hi, lo = _hi_lo(wfull)
    # w[p, 2k+0] = hi[k*128+p], w[p, 2k+1] = lo[k*128+p]
    w = np.empty((128, 2 * KC), dtype=ml_dtypes.bfloat16)
    w[:, 0::2] = hi.reshape(KC, 128).T
    w[:, 1::2] = lo.reshape(KC, 128).T
    tjv = np.ascontiguousarray(T.astype(np.float32).reshape(KC, 128).T)
    in_maps = []
    for c in range(NCORES):
        tib = np.ascontiguousarray(
            np.broadcast_to(T[c * LI : (c + 1) * LI].astype(np.float32), (128, LI))
        )
        in_maps.append({"tib": tib, "tj": tjv, "w": w})
    return in_maps, P_exp


def _sign_correction(P_exp, T):
    """Per-row correction recovering G from the ACT chunks' 0.5*(G-L):
    add 0.5*S_act - 0.5*Eq_i, with Eq_i the exact sum of P_exp over
    ACT-chunk j with T_j == T_i (self included)."""
    act_j = np.zeros(N, dtype=bool)
    for k in _act_chunks():
        act_j[k * 128 : (k + 1) * 128] = True
    S_act = np.float32(P_exp[act_j].sum(dtype=np.float64))
    uniq, inv = np.unique(T, return_inverse=True)
    eq_group = np.zeros(len(uniq), np.float32)
    np.add.at(eq_group, inv[act_j], P_exp[act_j])
    Eq = eq_group[inv]
    return np.float32(0.5) * S_act - np.float32(0.5) * Eq


def _epilogue(P_risk, T, E, P_exp, P_exp_sum):
    T = T.astype(np.float32)
    has_risk = (T < T.max()).astype(np.float32)
    Ef = E.astype(np.float32) * has_risk
    P_tmp = P_exp / (P_exp_sum + np.float32(EPS))
    upper = P_tmp.max()
    P_clipped = np.clip(P_tmp, np.float32(EPS), upper)
    loss = -np.sum(np.log(P_clipped) * Ef, dtype=np.float32) / np.sum(
        Ef, dtype=np.float32
    )
    return np.asarray(loss, dtype=np.float32)


def kernel(P_risk, T, E):
    from concourse.bass_utils import run_bass_kernel_spmd

    nc = _build_program()
    in_maps, P_exp = _make_in_maps(P_risk, T)
    corr = _sign_correction(P_exp, T.astype(np.float32))
    S_total = float(P_exp.sum(dtype=np.float64))
    last_err = None
    for _attempt in range(3):
        try:
            res = run_bass_kernel_spmd(nc, in_maps, core_ids=list(range(NCORES)))
            outs = np.stack([res.results[c]["out"] for c in range(NCORES)])
            g = (outs[:, 0, :] + outs[:, 1, :]).reshape(N)
            P_exp_sum = g + corr
            # sanity: each risk-set sum lies in [0, sum(P_exp)]; the row
            # holding max(T) has an empty risk set. Guards against a
            # silently-failed device execution.
            ok = (
                np.isfinite(P_exp_sum).all()
                and float(P_exp_sum.min()) >= -1e-2
                and float(P_exp_sum.max()) <= S_total * 1.001
                and abs(float(P_exp_sum[int(np.argmax(T))])) < 1e-2
                and float(P_exp_sum.max()) > 0.0
            )
            if ok:
                return _epilogue(P_risk, T, E, P_exp, P_exp_sum)
            last_err = RuntimeError("device output failed sanity check")
        except Exception as e:  # transient NRT device errors happen
            last_err = e
    raise last_err



# revision 6
# speedup vs baseline: 84.4315x; 84.4315x over previous
"""Cox partial-likelihood (DeepSurv) loss on 8 TRN2 NeuronCores.

Math: P_exp_sum[i] = sum_j P_exp[j] * (T[i] < T[j]); loss is the
Ef-weighted mean of -log(clip(P_exp / (P_exp_sum + eps), eps, max)).

The risk-set matrix M[i,j] = (T[i] < T[j]) is (up to ties) a
permutation of a strictly-upper-triangular matrix: in T-ascending
order the risk-set sum is a strict suffix sum of the sorted P_exp.
The host argsorts T (the previous full-mask kernel already relied on a
host-side sort via np.unique for its tie correction); the device then
computes the entire [N,N]-equivalent risk-set reduction AND the loss
epilogue exactly, data-parallel over 2048 sorted rows per core:

- rows are grouped into 128 blocks of 128 (16 blocks per core);
- within-block strict suffix sums: one [128,128] strictly-triangular
  matmul per core (stationary = the core's 16 P columns, moving = the
  triangular ones matrix);
- cross-block suffix: DVE reduces per-block totals, a [128,1]x[128,16]
  matmul forms per-block suffix sums, and a K=1 matmul broadcast-
  accumulates them into the same PSUM tile;
- exact tie handling: a host-computed per-row offset (EPS - corr_i,
  corr_i = sum of P_exp over later-sorted ties of i) is added on DVE,
  so the device result is G_i + EPS with strict-< semantics;
- epilogue on device: -log(P_clipped)*Ef per row via ACT Ln (ln P_exp
  == P_risk exactly, so -ln(P_tmp) = ln(G+eps) - P_risk; the lower
  clip at EPS becomes min(., -ln EPS); the upper clip at max(P_tmp) is
  a value no-op), reduced to a per-core partial numerator with a final
  ones-matmul over partitions. The host sums the 8 partial scalars and
  divides by sum(Ef).
"""

import numpy as np

N = 16384
NCORES = 8
NBLK = 128            # sorted-row blocks of 128
BPC = NBLK // NCORES  # blocks per core = 16
LI = N // NCORES      # rows per core = 2048
EPS = 1e-6
NEG_LN_EPS = float(-np.log(np.float32(EPS)))

# packed-input column layout (all fp32, [128, XC])
_C_PB = 0      # [128,128] pe_byblock[b, j] = P_s[b*128 + j]
_C_U1 = 128    # [128,128] U1[c, i] = 1.0 if c > i
_C_PC = 256    # [128,16]  pcore[p, k] = P_s[(blk0+k)*128 + p]
_C_UC = 272    # [128,16]  Ucore[b, k] = 1.0 if b > blk0 + k
_C_OF = 288    # [128,16]  off[p, k] = EPS - corr[(blk0+k)*128 + p]
_C_EF = 304    # [128,16]  Ef_s per core, same layout as pcore
_C_PR = 320    # [128,16]  P_risk_s per core, same layout
_C_OC = 336    # [128,1]   ones column
_C_OR = 337    # [128,128] ones (row 0 used as the K=1 broadcast lhsT)
XC = 465
NOUT = 18      # out cols: 0:16 g_eps, 16 row-partials, 17 scalar (row 0)

_prog_cache = {}


def _build_program(reps=1):
    if reps in _prog_cache:
        return _prog_cache[reps]
    import concourse.bacc as bacc
    import concourse.tile as tile
    import concourse.mybir as mybir

    f32 = mybir.dt.float32
    nc = bacc.Bacc(
        "TRN2", target_bir_lowering=False, debug=False, num_devices=NCORES
    )
    inp = nc.dram_tensor("inp", [128, XC], f32, kind="ExternalInput").ap()
    out = nc.dram_tensor("out", [128, NOUT], f32, kind="ExternalOutput").ap()

    with tile.TileContext(nc) as tc:
        with (
            tc.tile_pool(name="const", bufs=1) as cpool,
            tc.tile_pool(name="work", bufs=3) as wpool,
            tc.tile_pool(name="psa", bufs=2, space="PSUM") as pa,
            tc.tile_pool(name="psb", bufs=2, space="PSUM") as pb,
            tc.tile_pool(name="psc", bufs=2, space="PSUM") as pc,
        ):
            inp_s = cpool.tile([128, XC], f32)
            nc.sync.dma_start(inp_s[:], inp[:])
            scr = cpool.tile([128, 128], f32)
            res = cpool.tile([128, NOUT], f32)
            nc.vector.memset(res[:, 17:18], 0.0)

            ge = rs = sc_ps = None
            for _ in range(reps):
                # per-block totals -> S_suf
                totals = wpool.tile([128, 1], f32, name="totals", tag="tot")
                nc.vector.tensor_reduce(
                    totals[:],
                    inp_s[:, _C_PB : _C_PB + 128],
                    mybir.AxisListType.X,
                    mybir.AluOpType.add,
                )
                sr_ps = pa.tile([1, BPC], f32, name="sr_ps", tag="srp")
                nc.tensor.matmul(
                    sr_ps[:],
                    totals[:],
                    inp_s[:, _C_UC : _C_UC + BPC],
                    start=True,
                    stop=True,
                )
                srow = wpool.tile([1, BPC], f32, name="srow", tag="srow")
                nc.vector.tensor_copy(srow[:], sr_ps[:])
                # within-block strict suffix + broadcast S_suf, same PSUM
                g_ps = pb.tile([128, BPC], f32, name="g_ps", tag="gp")
                nc.tensor.matmul(
                    g_ps[:],
                    inp_s[:, _C_U1 : _C_U1 + 128],
                    inp_s[:, _C_PC : _C_PC + BPC],
                    start=True,
                    stop=False,
                )
                nc.tensor.matmul(
                    g_ps[:],
                    inp_s[0:1, _C_OR : _C_OR + 128],
                    srow[:],
                    start=False,
                    stop=True,
                )
                # g_eps = G + EPS (off folds EPS and the exact tie corr)
                ge = wpool.tile([128, BPC], f32, name="ge", tag="ge")
                nc.vector.tensor_tensor(
                    ge[:], g_ps[:], inp_s[:, _C_OF : _C_OF + BPC],
                    mybir.AluOpType.add,
                )
                # -ln(P_tmp) = ln(G+eps) - P_risk; lower clip -> min
                lng = wpool.tile([128, BPC], f32, name="lng", tag="lng")
                nc.scalar.activation(
                    lng[:], ge[:], mybir.ActivationFunctionType.Ln
                )
                d = wpool.tile([128, BPC], f32, name="d", tag="d")
                nc.vector.tensor_tensor(
                    d[:], lng[:], inp_s[:, _C_PR : _C_PR + BPC],
                    mybir.AluOpType.subtract,
                )
                dc = wpool.tile([128, BPC], f32, name="dc", tag="dc")
                nc.vector.tensor_scalar_min(dc[:], d[:], NEG_LN_EPS)
                mt = wpool.tile([128, BPC], f32, name="mt", tag="mt")
                nc.vector.tensor_tensor(
                    mt[:], dc[:], inp_s[:, _C_EF : _C_EF + BPC],
                    mybir.AluOpType.mult,
                )
                rs = wpool.tile([128, 1], f32, name="rs", tag="rs")
                nc.vector.tensor_reduce(
                    rs[:], mt[:], mybir.AxisListType.X, mybir.AluOpType.add,
                )
                sc_ps = pc.tile([1, 1], f32, name="sc_ps", tag="sc")
                nc.tensor.matmul(
                    sc_ps[:],
                    rs[:],
                    inp_s[:, _C_OC : _C_OC + 1],
                    start=True,
                    stop=True,
                )
            # export the last rep's results (constant cost, outside the body)
            nc.vector.tensor_copy(res[:, 0:16], ge[:])
            nc.vector.tensor_copy(res[:, 16:17], rs[:])
            nc.vector.tensor_copy(res[0:1, 17:18], sc_ps[:])
            nc.sync.dma_start(out[:], res[:])
    nc.compile()
    _prog_cache[reps] = nc
    return nc


def _tie_corr(T_s, P_s):
    """corr[i] = sum of P_s over later-sorted j with T_s[j] == T_s[i]
    (the device's index-strict suffix overcounts exactly this)."""
    corr = np.zeros(N, np.float32)
    neq = T_s[1:] != T_s[:-1]
    if neq.all():
        return corr
    starts = np.flatnonzero(np.concatenate(([True], neq)))
    lens = np.diff(np.append(starts, N))
    for st, ln in zip(starts[lens > 1], lens[lens > 1]):
        g = P_s[st : st + ln].astype(np.float64)
        sfx = np.cumsum(g[::-1])[::-1] - g
        corr[st : st + ln] = sfx.astype(np.float32)
    return corr


def _make_in_maps(P_risk, T, E):
    P_risk = P_risk.astype(np.float32)
    T = T.astype(np.float32)
    P_exp = np.exp(P_risk)
    Ef = E.astype(np.float32) * (T < T.max()).astype(np.float32)

    order = np.argsort(T, kind="stable")
    T_s = T[order]
    P_s = P_exp[order]
    Pr_s = P_risk[order]
    Ef_s = Ef[order]
    corr = _tie_corr(T_s, P_s)
    offv = np.float32(EPS) - corr

    pe_byblock = np.ascontiguousarray(P_s.reshape(NBLK, 128))
    u1 = np.greater.outer(np.arange(128), np.arange(128)).astype(np.float32)
    onescol = np.ones((128, 1), np.float32)
    onesrow = np.ones((128, 128), np.float32)

    def core_cols(v):  # sorted [N] -> per-core [128, BPC]
        return np.ascontiguousarray(v.reshape(NBLK, 128).T)

    pc_all = P_s.reshape(NBLK, 128)
    of_all = offv.reshape(NBLK, 128)
    ef_all = Ef_s.reshape(NBLK, 128)
    pr_all = Pr_s.reshape(NBLK, 128)

    in_maps = []
    for c in range(NCORES):
        b0 = c * BPC
        uc = np.greater.outer(
            np.arange(128), b0 + np.arange(BPC)
        ).astype(np.float32)
        blk = slice(b0, b0 + BPC)
        inp = np.concatenate(
            [
                pe_byblock,
                u1,
                np.ascontiguousarray(pc_all[blk].T),
                uc,
                np.ascontiguousarray(of_all[blk].T),
                np.ascontiguousarray(ef_all[blk].T),
                np.ascontiguousarray(pr_all[blk].T),
                onescol,
                onesrow,
            ],
            axis=1,
        )
        assert inp.shape == (128, XC) and inp.dtype == np.float32
        in_maps.append({"inp": inp})

    aux = {
        "P_exp": P_exp,
        "order": order,
        "corr": corr,
        "Ef": Ef,
        "P_s": P_s,
    }
    return in_maps, aux


def kernel(P_risk, T, E):
    from concourse.bass_utils import run_bass_kernel_spmd

    nc = _build_program()
    in_maps, aux = _make_in_maps(P_risk, T, E)
    denom = np.sum(aux["Ef"], dtype=np.float32)
    S_total = float(aux["P_exp"].sum(dtype=np.float64))
    last_err = None
    for _attempt in range(3):
        try:
            res = run_bass_kernel_spmd(nc, in_maps, core_ids=list(range(NCORES)))
            outs = np.stack([res.results[c]["out"] for c in range(NCORES)])
            partials = outs[:, 0, 17]
            # g_eps back to sorted order: core c col k row p -> (c*16+k)*128+p
            g_eps = np.transpose(outs[:, :, 0:16], (0, 2, 1)).reshape(N)
            s_dev = g_eps.astype(np.float64) - EPS + aux["corr"]
            # sanity: suffix sums are non-increasing in sorted order, start
            # near S_total, and the max-T row has an empty risk set.
            ok = (
                np.isfinite(outs).all()
                and float(np.max(np.diff(s_dev))) < 0.5
                and abs(s_dev[0] + aux["P_s"][0] - S_total) < 0.005 * S_total
                and abs(s_dev[-1]) < 1e-2
                and s_dev.min() > -1e-2
            )
            if ok:
                loss = np.float32(partials.sum(dtype=np.float64)) / denom
                return np.asarray(loss, dtype=np.float32)
            last_err = RuntimeError("device output failed sanity check")
        except Exception as e:  # transient NRT device errors happen
            last_err = e
    raise last_err
